# revision 3
# baseline (speedup 1.0000x reference)
"""GATv2Conv Trainium2 kernel (8-core SPMD, src-sharded edges).

Design:
  - Edges sharded by SRC range: core k owns srcs [k*12544, (k+1)*12544).
  - Phase 1 (per core): tab[r] = [h(64)|s_src(4)|junk] fp16 for OWN 12544
    nodes, rows PERMUTED partition-major per 7-window group so writes are
    128 x 1792B contiguous descriptors. s_dst goes to sdT [128, 392] fp16,
    AllGather'd (1.6MB total).
  - Phase 2: edges laid out by global dst window (784 windows, B cols of
    128); chunks of 14 windows: B dma_gather ops (1792 idx each, 256B
    rows, 4 SWDGE queues); one-hot matmuls expand s_dst (PE transpose
    path) and scatter [p*h | p] into PSUM; partial accumulator written
    block-contiguous (128 x 3808B descs) to acc [100352, 68] f32.
  - ReduceScatter(add): core k owns blocks [7k, 7k+7); final div
    num/(den+eps) -> out [12544, 64] (block-permuted; host unpermutes).
"""
import math
import os
import time
from contextlib import ExitStack
from dataclasses import dataclass

import numpy as np

import concourse.bass as bass
import concourse.bacc as bacc
import concourse.mybir as mybir
import concourse.tile as tile
from concourse import bass_utils

F32 = mybir.dt.float32
F16 = mybir.dt.float16
I16 = mybir.dt.int16

N_NODES = 100000
N_EDGES = 1600000
HEADS = 4
HEAD_DIM = 16
EPS = 1e-8
NEG = 0.2
IN_CH = 128
ELEM = 128      # fp16 elems per tab row (256B)

LAST_EXEC_NS = None
LAST_NC = None
LAST_IN_MAPS = None


@dataclass
class Cfg:
    cores: int = 8
    nloc: int = 12544          # nodes per core (src ownership, 128-aligned)
    wins: int = 784            # global dst windows (100352 rows)
    chw: int = 7               # windows per chunk (784/7=112; 98/7=14)
    p1g: int = 7               # windows per phase-1 group (98 = 14*7)

    @property
    def npad(self):
        return self.wins * 128  # 100352

    @property
    def npc(self):
        return self.nloc

    @property
    def nch(self):
        return self.wins // self.chw  # 112

    @property
    def lwins(self):
        return self.nloc // 128  # 98


def _make_ap(base_ap, rel_offset, dims):
    return bass.AP(base_ap.tensor, base_ap.offset + rel_offset,
                   [list(d) for d in dims])


def _bcast_dim(ap_obj, insert_at, count):
    newap = [list(x) for x in ap_obj.ap]
    newap.insert(insert_at, [0, count])
    return bass.AP(ap_obj.tensor, ap_obj.offset, newap)


def _host_prep(C, x, edge_index, edge_weight, W, a, B=3):
    src = np.asarray(edge_index[0], dtype=np.int64)
    dst = np.asarray(edge_index[1], dtype=np.int64)
    w = np.asarray(edge_weight, dtype=np.float32)

    core = np.minimum(src // C.nloc, C.cores - 1)
    win = dst >> 7
    dstc_v = (dst & 127).astype(np.float32)
    # local src id -> permuted tab row: r = (w//7)*896 + p*7 + (w%7)
    nl = src - core * C.nloc
    wl, pl = nl >> 7, nl & 127
    loc_src = (wl // C.p1g) * (C.p1g * 128) + pl * C.p1g + (wl % C.p1g)

    group = core * C.wins + win
    order = np.argsort(group, kind="stable")
    g_sorted = group[order]
    ngroups = C.cores * C.wins
    counts = np.bincount(g_sorted, minlength=ngroups)
    need_b = int(math.ceil(counts.max() / 128.0))
    if need_b > B:
        B = need_b

    starts = np.zeros(ngroups, dtype=np.int64)
    np.cumsum(counts[:-1], out=starts[1:])
    iw = np.arange(len(src), dtype=np.int64) - starts[g_sorted]

    core_s = g_sorted // C.wins
    win_s = g_sorted % C.wins
    rows = iw & 127
    cols = win_s * B + (iw >> 7)

    tcols = C.wins * B
    sh = (C.cores, 128, tcols)
    idxg = np.zeros(sh, dtype=np.int16)
    dstc = np.full(sh, -1.0, dtype=np.float32)
    wc = np.zeros(sh, dtype=np.float32)
    idxg[core_s, rows, cols] = loc_src[order].astype(np.int16)
    dstc[core_s, rows, cols] = dstc_v[order]
    wc[core_s, rows, cols] = w[order]

    # pack idx for dma_gather: flat position i = col*128 + row ->
    # packed[p % 16, i // 16]; replicate x8 across partition groups
    idxp = np.empty((C.cores, 128, tcols * 8), dtype=np.int16)
    for c in range(C.cores):
        flat = idxg[c].T.ravel()
        arr = flat.reshape(-1, 16).T
        idxp[c] = np.tile(arr, (8, 1))

    xf = np.asarray(x, dtype=np.float32)
    xts = []
    for c in range(C.cores):
        lo = c * C.nloc
        hi = min((c + 1) * C.nloc, N_NODES)
        xt = np.zeros((IN_CH, C.nloc), dtype=np.float32)
        xt[:, :hi - lo] = xf[lo:hi].T
        xts.append(xt)

    Wt = np.ascontiguousarray(np.asarray(W, dtype=np.float32).T)  # [128,64]
    a_np = np.asarray(a, dtype=np.float32)
    a_src = a_np[0, :, :HEAD_DIM]
    a_dst = a_np[0, :, HEAD_DIM:]
    A_src = (Wt.reshape(IN_CH, HEADS, HEAD_DIM) * a_src[None]).sum(-1)
    A_dst = (Wt.reshape(IN_CH, HEADS, HEAD_DIM) * a_dst[None]).sum(-1)
    rhs_ext = np.ascontiguousarray(
        np.concatenate([Wt, A_src, A_dst], axis=1), dtype=np.float32)
    iota = np.ascontiguousarray(np.broadcast_to(
        np.arange(128, dtype=np.float32), (128, 128)))
    ident = np.eye(128, dtype=np.float32)

    in_maps = []
    for c in range(C.cores):
        in_maps.append(dict(
            xT=xts[c], rhs_ext=rhs_ext, iota=iota, ident=ident,
            idxp=idxp[c], dstc=dstc[c], wc=wc[c]))
    return in_maps, B


def _build_program(C, B, num_devices=None):
    OHD = F32 if os.environ.get("K2_F32OH", "1") == "1" else F16
    REPS = int(os.environ.get("K2_REPS", "1"))
    REP1 = int(os.environ.get("K2_REP1", "1"))
    REPCC = int(os.environ.get("K2_REPCC", "1"))
    ND = num_devices or C.cores
    Kc = C.chw * B
    tcols = C.wins * B
    opk = int(os.environ.get("K2_OPK", "7"))   # columns per gather op
    assert Kc % opk == 0
    nidx = 128 * opk
    n_go = Kc // opk

    nc = bacc.Bacc("TRN2", target_bir_lowering=False, debug=False,
                   enable_asserts=False, num_devices=ND,
                   dynamic_dma_scratch_size=32768, num_swdge_queues=4)
    xT_d = nc.dram_tensor("xT", [IN_CH, C.nloc], F32, kind="ExternalInput")
    re_d = nc.dram_tensor("rhs_ext", [IN_CH, 72], F32, kind="ExternalInput")
    io_d = nc.dram_tensor("iota", [128, 128], F32, kind="ExternalInput")
    id_d = nc.dram_tensor("ident", [128, 128], F32, kind="ExternalInput")
    idxp_d = nc.dram_tensor("idxp", [128, tcols * 8], I16,
                            kind="ExternalInput")
    dstc_d = nc.dram_tensor("dstc", [128, tcols], F32, kind="ExternalInput")
    wc_d = nc.dram_tensor("wc", [128, tcols], F32, kind="ExternalInput")

    tab_d = nc.dram_tensor("tab", [C.nloc, ELEM], F32, kind="Internal")
    sdT_d = nc.dram_tensor("sdT", [128, C.lwins * 4], F32, kind="Internal")
    sdF_d = nc.dram_tensor("sdF", [128 * ND, C.lwins * 4], F32,
                           kind="Internal", addr_space="Shared")
    acc_d = nc.dram_tensor("acc", [C.npad, 68], F32, kind="Internal")
    red_d = nc.dram_tensor("red", [C.nloc, 68], F32, kind="Internal")
    out_d = nc.dram_tensor("out", [C.nloc, 64], F32, kind="ExternalOutput")

    groups = [list(range(ND))]

    with tile.TileContext(nc) as tc, ExitStack() as ctx:
        const = ctx.enter_context(tc.tile_pool(name="const", bufs=1))
        iota_f = const.tile([128, 128], F32)
        nc.sync.dma_start(out=iota_f[:], in_=io_d[:])
        id_f = const.tile([128, 128], F32)
        nc.sync.dma_start(out=id_f[:], in_=id_d[:])
        iota_t = const.tile([128, 128], OHD)
        nc.vector.tensor_copy(out=iota_t[:], in_=iota_f[:])
        id_t = const.tile([128, 128], OHD)
        nc.vector.tensor_copy(out=id_t[:], in_=id_f[:])
        re_t = const.tile([128, 72], F32)
        nc.sync.dma_start(out=re_t[:], in_=re_d[:])
        dstc_f = const.tile([128, tcols], F32)
        nc.sync.dma_start(out=dstc_f[:], in_=dstc_d[:])
        dstc_t = const.tile([128, tcols], OHD)
        nc.vector.tensor_copy(out=dstc_t[:], in_=dstc_f[:])
        wc_t = const.tile([128, tcols], F32)
        nc.sync.dma_start(out=wc_t[:], in_=wc_d[:])
        sdall = const.tile([128, ND, C.lwins * 4], F32)

        # ---- phase 1: tab rows (permuted) = [h | s_src | junk] fp16 ----
        PG = C.p1g
        with tc.tile_pool(name="xload", bufs=1) as xp, \
             tc.tile_pool(name="hstage", bufs=3) as hp, \
             tc.tile_pool(name="psh", bufs=2, space="PSUM") as php:
          for _r1 in range(REP1):
            xt = xp.tile([128, C.nloc], F32, tag="xt")
            nc.sync.dma_start(out=xt[:], in_=xT_d[:])
            sdstage = xp.tile([128, C.lwins, 4], F32, tag="sds")
            for g7 in range(C.lwins // PG):
                ph = php.tile([128, PG, 72], F32, tag="ph")
                for j in range(PG):
                    jw = g7 * PG + j
                    nc.tensor.matmul(
                        out=ph[:, j, :], lhsT=xt[:, jw * 128:(jw + 1) * 128],
                        rhs=re_t[:], start=True, stop=True)
                hs = hp.tile([128, PG, ELEM], F32, tag="hs")
                nc.vector.tensor_copy(out=hs[:, :, 0:68],
                                      in_=ph[:, :, 0:68])
                nc.vector.tensor_copy(
                    out=sdstage[:, g7 * PG:(g7 + 1) * PG, :],
                    in_=ph[:, :, 68:72])
                dst_ap = _make_ap(
                    tab_d[:], g7 * PG * 128 * ELEM,
                    [[PG * ELEM, 128], [1, PG * ELEM]])
                nc.sync.dma_start(
                    out=dst_ap, in_=hs[:].rearrange("p j e -> p (j e)"))
            nc.sync.dma_start(
                out=sdT_d[:],
                in_=sdstage[:].rearrange("p j e -> p (j e)"))

        nc.gpsimd.collective_compute(
            "AllGather", mybir.AluOpType.bypass, groups,
            ins=[sdT_d[:]], outs=[sdF_d[:]])
        src_ap = _make_ap(sdF_d[:], 0,
                          [[C.lwins * 4, 128], [128 * C.lwins * 4, ND],
                           [1, C.lwins * 4]])
        nc.sync.dma_start(out=sdall[:], in_=src_ap)

        # ---------------- phase 2: edges ----------------
        sb = ctx.enter_context(tc.tile_pool(name="edge", bufs=2))
        wb = ctx.enter_context(tc.tile_pool(name="winb", bufs=2))
        pst = ctx.enter_context(tc.tile_pool(name="pst", bufs=2,
                                             space="PSUM"))
        pss = ctx.enter_context(tc.tile_pool(name="pss", bufs=2,
                                             space="PSUM"))
        psa = ctx.enter_context(tc.tile_pool(name="psa", bufs=2,
                                             space="PSUM"))

        def emit_oh(c):
            oh = wb.tile([128, Kc, 128], OHD, tag="oh")
            nc.vector.tensor_tensor(
                out=oh[:], in0=_bcast_dim(iota_t[:], 1, Kc),
                in1=dstc_t[:, c * Kc:(c + 1) * Kc].to_broadcast(
                    [128, Kc, 128]),
                op=mybir.AluOpType.is_equal)
            return oh

        def emit_gather(c):
            idx_t = sb.tile([128, Kc * 8], I16, tag="idx")
            nc.sync.dma_start(out=idx_t[:],
                              in_=idxp_d[:, c * Kc * 8:(c + 1) * Kc * 8])
            g = sb.tile([128, Kc, ELEM], F32, tag="g")
            for o in range(n_go):
                nc.gpsimd.dma_gather(
                    g[:, o * opk:(o + 1) * opk, :], tab_d[:],
                    idx_t[:, o * (nidx // 16):(o + 1) * (nidx // 16)],
                    nidx, nidx, ELEM, queue_num=(c * n_go + o) % 4)
            return g

        def emit_trans(c, oh):
            ohT = wb.tile([128, Kc, 128], OHD, tag="ohT")
            for t in range(0, Kc, 4):
                tw = min(4, Kc - t)
                psT = pst.tile([128, 4, 128], OHD, tag="psT")
                for j4 in range(tw):
                    nc.tensor.transpose(out=psT[:, j4, :],
                                        in_=oh[:, t + j4, :],
                                        identity=id_t[:])
                nc.vector.tensor_copy(out=ohT[:, t:t + tw, :],
                                      in_=psT[:, 0:tw, :])
            sde_ps = pss.tile([128, Kc, 4], F32, tag="sde")
            for col in range(Kc):
                wg = c * C.chw + col // B
                k8, j98 = divmod(wg, C.lwins)
                nc.tensor.matmul(
                    out=sde_ps[:, col, :], lhsT=ohT[:, col, :],
                    rhs=sdall[:, k8, j98 * 4:(j98 + 1) * 4],
                    start=True, stop=True)
            return sde_ps

        def emit_back(c, g, oh, sde_ps):
            logit = wb.tile([128, Kc, 4], F32, tag="logit")
            nc.vector.tensor_add(
                out=logit[:], in0=g[:, :, 64:68], in1=sde_ps[:])
            nc.vector.scalar_tensor_tensor(
                out=logit[:], in0=logit[:], scalar=NEG, in1=logit[:],
                op0=mybir.AluOpType.mult, op1=mybir.AluOpType.max)
            nc.vector.tensor_mul(
                out=logit[:], in0=logit[:],
                in1=wc_t[:, c * Kc:(c + 1) * Kc].to_broadcast([128, Kc, 4]))
            p = wb.tile([128, Kc, 4], F32, tag="p")
            nc.scalar.activation(p[:], logit[:],
                                 mybir.ActivationFunctionType.Exp)

            pay = wb.tile([128, Kc, 68], F32, tag="pay")
            pv = p[:].to_broadcast([128, Kc, 4, 16])
            gv = g[:, :, 0:64].rearrange("p k (h d) -> p k h d", d=16)
            ov = pay[:, :, 0:64].rearrange("p k (h d) -> p k h d", d=16)
            nc.vector.tensor_mul(out=ov, in0=gv, in1=pv)
            nc.vector.tensor_copy(out=pay[:, :, 64:68], in_=p[:])

            stage = wb.tile([128, C.chw, 68], F32, tag="stage")
            for gq in range(0, C.chw, PG):
                acc_ps = psa.tile([128, PG, 68], F32, tag="acc")
                for wl in range(PG):
                    c0 = (gq + wl) * B
                    for j in range(B):
                        nc.tensor.matmul(
                            out=acc_ps[:, wl, :], lhsT=oh[:, c0 + j, :],
                            rhs=pay[:, c0 + j, :],
                            start=(j == 0), stop=(j == B - 1))
                nc.vector.tensor_copy(out=stage[:, gq:gq + PG, :],
                                      in_=acc_ps[:])
            dst_ap = _make_ap(
                acc_d[:], c * C.chw * 128 * 68,
                [[C.chw * 68, 128], [1, C.chw * 68]])
            nc.sync.dma_start(
                out=dst_ap, in_=stage[:].rearrange("p w e -> p (w e)"))

        for _rp in range(REPS):
            for c in range(C.nch):
                g = emit_gather(c)
                oh = emit_oh(c)
                sde_ps = emit_trans(c, oh)
                emit_back(c, g, oh, sde_ps)

        for _rc in range(REPCC):
            nc.gpsimd.collective_compute(
                "ReduceScatter", mybir.AluOpType.add, groups,
                ins=[acc_d[:]], outs=[red_d[:]])

        # ---------------- final: out = num / (den + eps) ----------------
        FW = C.chw
        with tc.tile_pool(name="fin", bufs=2) as fp:
            for fb in range(C.lwins // FW):
                src2 = _make_ap(red_d[:], fb * FW * 128 * 68,
                                [[FW * 68, 128], [1, FW * 68]])
                rt = fp.tile([128, FW, 68], F32, tag="rt")
                nc.sync.dma_start(
                    out=rt[:].rearrange("p w e -> p (w e)"), in_=src2)
                rec = fp.tile([128, FW, 4], F32, tag="rec")
                nc.vector.tensor_scalar_add(out=rec[:], in0=rt[:, :, 64:68],
                                            scalar1=EPS)
                nc.vector.reciprocal(out=rec[:], in_=rec[:])
                ot = fp.tile([128, FW, 64], F32, tag="ot")
                nc.vector.tensor_mul(
                    out=ot[:].rearrange("p k (h d) -> p k h d", d=16),
                    in0=rt[:, :, 0:64].rearrange("p k (h d) -> p k h d",
                                                 d=16),
                    in1=rec[:].to_broadcast([128, FW, 4, 16]))
                dst2 = _make_ap(out_d[:], fb * FW * 128 * 64,
                                [[FW * 64, 128], [1, FW * 64]])
                nc.sync.dma_start(
                    out=dst2, in_=ot[:].rearrange("p w e -> p (w e)"))

    nc.compile()
    return nc


def _build_base(C, B, num_devices=None):
    """I/O-identical near-empty program for dispatch-overhead calibration."""
    ND = num_devices or C.cores
    tcols = C.wins * B
    nc = bacc.Bacc("TRN2", target_bir_lowering=False, debug=False,
                   enable_asserts=False, num_devices=ND)
    nc.dram_tensor("xT", [IN_CH, C.nloc], F32, kind="ExternalInput")
    nc.dram_tensor("rhs_ext", [IN_CH, 72], F32, kind="ExternalInput")
    io_d = nc.dram_tensor("iota", [128, 128], F32, kind="ExternalInput")
    nc.dram_tensor("ident", [128, 128], F16, kind="ExternalInput")
    nc.dram_tensor("idxp", [128, tcols * 8], I16, kind="ExternalInput")
    nc.dram_tensor("dstc", [128, tcols], F16, kind="ExternalInput")
    nc.dram_tensor("wc", [128, tcols], F16, kind="ExternalInput")
    out_d = nc.dram_tensor("out", [C.nloc, 64], F32, kind="ExternalOutput")
    with tile.TileContext(nc) as tc, ExitStack() as ctx:
        sb = ctx.enter_context(tc.tile_pool(name="sb", bufs=1))
        f = sb.tile([128, 64], F32)
        nc.sync.dma_start(out=f[:], in_=io_d[:, 0:64])
        dst = _make_ap(out_d[:], 0, [[64, 128], [0, C.lwins], [1, 64]])
        nc.sync.dma_start(out=dst, in_=_bcast_dim(f[:], 1, C.lwins))
    nc.compile()
    return nc


def _unpermute_out(C, arr):
    """[12544, 64] block-permuted -> node order."""
    nb = C.lwins // C.chw  # 7
    return arr.reshape(nb, 128, C.chw, 64).transpose(
        0, 2, 1, 3).reshape(C.nloc, 64)


def kernel(x, edge_index, edge_weight, W, a):
    global LAST_EXEC_NS, LAST_NC, LAST_IN_MAPS
    C = Cfg()
    t0 = time.time()
    in_maps, B = _host_prep(C, x, edge_index, edge_weight, W, a)
    t1 = time.time()
    nc = _build_program(C, B)
    LAST_NC = nc
    LAST_IN_MAPS = in_maps
    t2 = time.time()
    res = bass_utils.run_bass_kernel_spmd(
        nc, in_maps, core_ids=list(range(C.cores)))
    t3 = time.time()
    print(f"[kernel2] host_prep {t1-t0:.1f}s  build+compile {t2-t1:.1f}s  "
          f"exec(all-in) {t3-t2:.1f}s  B={B}")
    LAST_EXEC_NS = res.exec_time_ns
    parts = [_unpermute_out(C, res.results[c]["out"])
             for c in range(C.cores)]
    full = np.concatenate(parts, axis=0)[:N_NODES]
    return np.ascontiguousarray(full)


# revision 4
# speedup vs baseline: 1.6027x; 1.6027x over previous
"""GATv2Conv Trainium2 kernel (8-core SPMD, src-sharded edges).

Design:
  - Edges sharded by SRC range: core k owns srcs [k*12544, (k+1)*12544).
  - Phase 1 (per core): tab[r] = [h(64)|s_src(4)|junk] fp16 for OWN 12544
    nodes, rows PERMUTED partition-major per 7-window group so writes are
    128 x 1792B contiguous descriptors. s_dst goes to sdT [128, 392] fp16,
    AllGather'd (1.6MB total).
  - Phase 2: edges laid out by global dst window (784 windows, B cols of
    128); chunks of 14 windows: B dma_gather ops (1792 idx each, 256B
    rows, 4 SWDGE queues); one-hot matmuls expand s_dst (PE transpose
    path) and scatter [p*h | p] into PSUM; partial accumulator written
    block-contiguous (128 x 3808B descs) to acc [100352, 68] f32.
  - ReduceScatter(add): core k owns blocks [7k, 7k+7); final div
    num/(den+eps) -> out [12544, 64] (block-permuted; host unpermutes).
"""
import math
import os
import time
from contextlib import ExitStack
from dataclasses import dataclass

import numpy as np

import concourse.bass as bass
import concourse.bacc as bacc
import concourse.mybir as mybir
import concourse.tile as tile
from concourse import bass_utils

F32 = mybir.dt.float32
F16 = mybir.dt.float16
I16 = mybir.dt.int16

N_NODES = 100000
N_EDGES = 1600000
HEADS = 4
HEAD_DIM = 16
EPS = 1e-8
NEG = 0.2
IN_CH = 128
ELEM = 128      # fp16 elems per tab row (256B)

LAST_EXEC_NS = None
LAST_NC = None
LAST_IN_MAPS = None


@dataclass
class Cfg:
    cores: int = 8
    nloc: int = 12544          # nodes per core (src ownership, 128-aligned)
    wins: int = 784            # global dst windows (100352 rows)
    chw: int = 7               # windows per chunk (784/7=112; 98/7=14)
    p1g: int = 7               # windows per phase-1 group (98 = 14*7)

    @property
    def npad(self):
        return self.wins * 128  # 100352

    @property
    def npc(self):
        return self.nloc

    @property
    def nch(self):
        return self.wins // self.chw  # 112

    @property
    def lwins(self):
        return self.nloc // 128  # 98


def _make_ap(base_ap, rel_offset, dims):
    return bass.AP(base_ap.tensor, base_ap.offset + rel_offset,
                   [list(d) for d in dims])


def _bcast_dim(ap_obj, insert_at, count):
    newap = [list(x) for x in ap_obj.ap]
    newap.insert(insert_at, [0, count])
    return bass.AP(ap_obj.tensor, ap_obj.offset, newap)


def _host_prep(C, x, edge_index, edge_weight, W, a, B=3):
    src = np.asarray(edge_index[0], dtype=np.int64)
    dst = np.asarray(edge_index[1], dtype=np.int64)
    w = np.asarray(edge_weight, dtype=np.float32)

    core = np.minimum(src // C.nloc, C.cores - 1)
    win = dst >> 7
    dstc_v = (dst & 127).astype(np.float32)
    # local src id -> permuted tab row: r = (w//7)*896 + p*7 + (w%7)
    nl = src - core * C.nloc
    wl, pl = nl >> 7, nl & 127
    loc_src = (wl // C.p1g) * (C.p1g * 128) + pl * C.p1g + (wl % C.p1g)

    group = core * C.wins + win
    order = np.argsort(group, kind="stable")
    g_sorted = group[order]
    ngroups = C.cores * C.wins
    counts = np.bincount(g_sorted, minlength=ngroups)
    need_b = int(math.ceil(counts.max() / 128.0))
    if need_b > B:
        B = need_b

    starts = np.zeros(ngroups, dtype=np.int64)
    np.cumsum(counts[:-1], out=starts[1:])
    iw = np.arange(len(src), dtype=np.int64) - starts[g_sorted]

    core_s = g_sorted // C.wins
    win_s = g_sorted % C.wins
    rows = iw & 127
    cols = win_s * B + (iw >> 7)

    tcols = C.wins * B
    sh = (C.cores, 128, tcols)
    idxg = np.zeros(sh, dtype=np.int16)
    dstc = np.full(sh, -1.0, dtype=np.float32)
    wc = np.zeros(sh, dtype=np.float32)
    idxg[core_s, rows, cols] = loc_src[order].astype(np.int16)
    dstc[core_s, rows, cols] = dstc_v[order]
    wc[core_s, rows, cols] = w[order]

    # pack idx for dma_gather: flat position i = col*128 + row ->
    # packed[p % 16, i // 16]; replicate x8 across partition groups
    idxp = np.empty((C.cores, 128, tcols * 8), dtype=np.int16)
    for c in range(C.cores):
        flat = idxg[c].T.ravel()
        arr = flat.reshape(-1, 16).T
        idxp[c] = np.tile(arr, (8, 1))

    xf = np.asarray(x, dtype=np.float32)
    xts = []
    for c in range(C.cores):
        lo = c * C.nloc
        hi = min((c + 1) * C.nloc, N_NODES)
        xt = np.zeros((IN_CH, C.nloc), dtype=np.float32)
        xt[:, :hi - lo] = xf[lo:hi].T
        xts.append(xt)

    Wt = np.ascontiguousarray(np.asarray(W, dtype=np.float32).T)  # [128,64]
    a_np = np.asarray(a, dtype=np.float32)
    a_src = a_np[0, :, :HEAD_DIM]
    a_dst = a_np[0, :, HEAD_DIM:]
    A_src = (Wt.reshape(IN_CH, HEADS, HEAD_DIM) * a_src[None]).sum(-1)
    A_dst = (Wt.reshape(IN_CH, HEADS, HEAD_DIM) * a_dst[None]).sum(-1)
    rhs_ext = np.ascontiguousarray(
        np.concatenate([Wt, A_src, A_dst], axis=1), dtype=np.float32)
    iota = np.ascontiguousarray(np.broadcast_to(
        np.arange(128, dtype=np.float32), (128, 128)))
    ident = np.eye(128, dtype=np.float32)

    in_maps = []
    for c in range(C.cores):
        in_maps.append(dict(
            xT=xts[c], rhs_ext=rhs_ext, iota=iota, ident=ident,
            idxp=idxp[c], dstc=dstc[c], wc=wc[c]))
    return in_maps, B


def _build_program(C, B, num_devices=None):
    OHD = F32 if os.environ.get("K2_F32OH", "1") == "1" else F16
    REPS = int(os.environ.get("K2_REPS", "1"))
    REP1 = int(os.environ.get("K2_REP1", "1"))
    REPCC = int(os.environ.get("K2_REPCC", "1"))
    ND = num_devices or C.cores
    Kc = C.chw * B
    tcols = C.wins * B
    opk = int(os.environ.get("K2_OPK", "7"))   # columns per gather op
    assert Kc % opk == 0
    nidx = 128 * opk
    n_go = Kc // opk

    nc = bacc.Bacc("TRN2", target_bir_lowering=False, debug=False,
                   enable_asserts=False, num_devices=ND,
                   dynamic_dma_scratch_size=32768, num_swdge_queues=4)
    xT_d = nc.dram_tensor("xT", [IN_CH, C.nloc], F32, kind="ExternalInput")
    re_d = nc.dram_tensor("rhs_ext", [IN_CH, 72], F32, kind="ExternalInput")
    io_d = nc.dram_tensor("iota", [128, 128], F32, kind="ExternalInput")
    id_d = nc.dram_tensor("ident", [128, 128], F32, kind="ExternalInput")
    idxp_d = nc.dram_tensor("idxp", [128, tcols * 8], I16,
                            kind="ExternalInput")
    dstc_d = nc.dram_tensor("dstc", [128, tcols], F32, kind="ExternalInput")
    wc_d = nc.dram_tensor("wc", [128, tcols], F32, kind="ExternalInput")

    tab_d = nc.dram_tensor("tab", [C.nloc, ELEM], F32, kind="Internal")
    sdT_d = nc.dram_tensor("sdT", [128, C.lwins * 4], F32, kind="Internal")
    sdF_d = nc.dram_tensor("sdF", [128 * ND, C.lwins * 4], F32,
                           kind="Internal", addr_space="Shared")
    acc_d = nc.dram_tensor("acc", [C.npad, 68], F32, kind="Internal")
    red_d = nc.dram_tensor("red", [C.nloc, 68], F32, kind="Internal")
    out_d = nc.dram_tensor("out", [C.nloc, 64], F32, kind="ExternalOutput")

    groups = [list(range(ND))]

    with tile.TileContext(nc) as tc, ExitStack() as ctx:
        const = ctx.enter_context(tc.tile_pool(name="const", bufs=1))
        iota_f = const.tile([128, 128], F32)
        nc.sync.dma_start(out=iota_f[:], in_=io_d[:])
        id_f = const.tile([128, 128], F32)
        nc.sync.dma_start(out=id_f[:], in_=id_d[:])
        iota_t = const.tile([128, 128], OHD)
        nc.vector.tensor_copy(out=iota_t[:], in_=iota_f[:])
        id_t = const.tile([128, 128], OHD)
        nc.vector.tensor_copy(out=id_t[:], in_=id_f[:])
        re_t = const.tile([128, 72], F32)
        nc.sync.dma_start(out=re_t[:], in_=re_d[:])
        dstc_f = const.tile([128, tcols], F32)
        nc.sync.dma_start(out=dstc_f[:], in_=dstc_d[:])
        dstc_t = const.tile([128, tcols], OHD)
        nc.vector.tensor_copy(out=dstc_t[:], in_=dstc_f[:])
        wc_t = const.tile([128, tcols], F32)
        nc.sync.dma_start(out=wc_t[:], in_=wc_d[:])
        sdall = const.tile([128, ND, C.lwins * 4], F32)

        # ---- phase 1: tab rows (permuted) = [h | s_src | junk] fp16 ----
        PG = C.p1g
        with tc.tile_pool(name="xload", bufs=1) as xp, \
             tc.tile_pool(name="hstage", bufs=3) as hp, \
             tc.tile_pool(name="psh", bufs=2, space="PSUM") as php:
          for _r1 in range(REP1):
            xt = xp.tile([128, C.nloc], F32, tag="xt")
            nc.sync.dma_start(out=xt[:], in_=xT_d[:])
            sdstage = xp.tile([128, C.lwins, 4], F32, tag="sds")
            for g7 in range(C.lwins // PG):
                ph = php.tile([128, PG, 72], F32, tag="ph")
                for j in range(PG):
                    jw = g7 * PG + j
                    nc.tensor.matmul(
                        out=ph[:, j, :], lhsT=xt[:, jw * 128:(jw + 1) * 128],
                        rhs=re_t[:], start=True, stop=True)
                hs = hp.tile([128, PG, ELEM], F32, tag="hs")
                nc.vector.tensor_copy(out=hs[:, :, 0:68],
                                      in_=ph[:, :, 0:68])
                nc.vector.tensor_copy(
                    out=sdstage[:, g7 * PG:(g7 + 1) * PG, :],
                    in_=ph[:, :, 68:72])
                dst_ap = _make_ap(
                    tab_d[:], g7 * PG * 128 * ELEM,
                    [[PG * ELEM, 128], [1, PG * ELEM]])
                nc.sync.dma_start(
                    out=dst_ap, in_=hs[:].rearrange("p j e -> p (j e)"))
            nc.sync.dma_start(
                out=sdT_d[:],
                in_=sdstage[:].rearrange("p j e -> p (j e)"))

        nc.gpsimd.collective_compute(
            "AllGather", mybir.AluOpType.bypass, groups,
            ins=[sdT_d[:]], outs=[sdF_d[:]])
        src_ap = _make_ap(sdF_d[:], 0,
                          [[C.lwins * 4, 128], [128 * C.lwins * 4, ND],
                           [1, C.lwins * 4]])
        nc.sync.dma_start(out=sdall[:], in_=src_ap)

        # ---------------- phase 2: edges ----------------
        sb = ctx.enter_context(tc.tile_pool(name="edge", bufs=2))
        wb = ctx.enter_context(tc.tile_pool(name="winb", bufs=2))
        pst = ctx.enter_context(tc.tile_pool(name="pst", bufs=2,
                                             space="PSUM"))
        pss = ctx.enter_context(tc.tile_pool(name="pss", bufs=2,
                                             space="PSUM"))
        psa = ctx.enter_context(tc.tile_pool(name="psa", bufs=2,
                                             space="PSUM"))

        def emit_oh(c):
            oh = wb.tile([128, Kc, 128], OHD, tag="oh")
            nc.vector.tensor_tensor(
                out=oh[:], in0=_bcast_dim(iota_t[:], 1, Kc),
                in1=dstc_t[:, c * Kc:(c + 1) * Kc].to_broadcast(
                    [128, Kc, 128]),
                op=mybir.AluOpType.is_equal)
            return oh

        def emit_gather(c):
            idx_t = sb.tile([128, Kc * 8], I16, tag="idx")
            nc.sync.dma_start(out=idx_t[:],
                              in_=idxp_d[:, c * Kc * 8:(c + 1) * Kc * 8])
            g = sb.tile([128, Kc, ELEM], F32, tag="g")
            for o in range(n_go):
                nc.gpsimd.dma_gather(
                    g[:, o * opk:(o + 1) * opk, :], tab_d[:],
                    idx_t[:, o * (nidx // 16):(o + 1) * (nidx // 16)],
                    nidx, nidx, ELEM, queue_num=(c * n_go + o) % 4)
            return g

        def emit_trans(c, oh):
            ohT = wb.tile([128, Kc, 128], OHD, tag="ohT")
            for t in range(0, Kc, 4):
                tw = min(4, Kc - t)
                psT = pst.tile([128, 4, 128], OHD, tag="psT")
                for j4 in range(tw):
                    nc.tensor.transpose(out=psT[:, j4, :],
                                        in_=oh[:, t + j4, :],
                                        identity=id_t[:])
                nc.vector.tensor_copy(out=ohT[:, t:t + tw, :],
                                      in_=psT[:, 0:tw, :])
            sde_ps = pss.tile([128, Kc, 4], F32, tag="sde")
            for col in range(Kc):
                wg = c * C.chw + col // B
                k8, j98 = divmod(wg, C.lwins)
                nc.tensor.matmul(
                    out=sde_ps[:, col, :], lhsT=ohT[:, col, :],
                    rhs=sdall[:, k8, j98 * 4:(j98 + 1) * 4],
                    start=True, stop=True)
            return sde_ps

        def emit_back(c, g, oh, sde_ps):
            logit = wb.tile([128, Kc, 4], F32, tag="logit")
            nc.vector.tensor_add(
                out=logit[:], in0=g[:, :, 64:68], in1=sde_ps[:])
            nc.vector.scalar_tensor_tensor(
                out=logit[:], in0=logit[:], scalar=NEG, in1=logit[:],
                op0=mybir.AluOpType.mult, op1=mybir.AluOpType.max)
            nc.vector.tensor_mul(
                out=logit[:], in0=logit[:],
                in1=wc_t[:, c * Kc:(c + 1) * Kc].to_broadcast([128, Kc, 4]))
            p = wb.tile([128, Kc, 4], F32, tag="p")
            nc.scalar.activation(p[:], logit[:],
                                 mybir.ActivationFunctionType.Exp)

            pay = wb.tile([128, Kc, 68], F32, tag="pay")
            pv = p[:].to_broadcast([128, Kc, 4, 16])
            gv = g[:, :, 0:64].rearrange("p k (h d) -> p k h d", d=16)
            ov = pay[:, :, 0:64].rearrange("p k (h d) -> p k h d", d=16)
            nc.vector.tensor_mul(out=ov, in0=gv, in1=pv)
            nc.vector.tensor_copy(out=pay[:, :, 64:68], in_=p[:])

            stage = wb.tile([128, C.chw, 68], F32, tag="stage")
            for gq in range(0, C.chw, PG):
                acc_ps = psa.tile([128, PG, 68], F32, tag="acc")
                for wl in range(PG):
                    c0 = (gq + wl) * B
                    for j in range(B):
                        nc.tensor.matmul(
                            out=acc_ps[:, wl, :], lhsT=oh[:, c0 + j, :],
                            rhs=pay[:, c0 + j, :],
                            start=(j == 0), stop=(j == B - 1))
                nc.vector.tensor_copy(out=stage[:, gq:gq + PG, :],
                                      in_=acc_ps[:])
            dst_ap = _make_ap(
                acc_d[:], c * C.chw * 128 * 68,
                [[C.chw * 68, 128], [1, C.chw * 68]])
            nc.sync.dma_start(
                out=dst_ap, in_=stage[:].rearrange("p w e -> p (w e)"))

        for _rp in range(REPS):
            for c in range(C.nch):
                g = emit_gather(c)
                oh = emit_oh(c)
                sde_ps = emit_trans(c, oh)
                emit_back(c, g, oh, sde_ps)

        SEGB = 8 * C.chw * 128           # rows per segment (8 blocks)
        OUTB = C.chw * 128                # rows per out block
        nseg = C.npad // SEGB             # 14
        for _rc in range(REPCC):
            if os.environ.get("K2_SEGRS", "1") == "1":
                for s in range(nseg):
                    nc.gpsimd.collective_compute(
                        "ReduceScatter", mybir.AluOpType.add, groups,
                        ins=[acc_d[s * SEGB:(s + 1) * SEGB, :]],
                        outs=[red_d[s * OUTB:(s + 1) * OUTB, :]])
            else:
                nc.gpsimd.collective_compute(
                    "ReduceScatter", mybir.AluOpType.add, groups,
                    ins=[acc_d[:]], outs=[red_d[:]])

        # ---------------- final: out = num / (den + eps) ----------------
        FW = C.chw
        with tc.tile_pool(name="fin", bufs=2) as fp:
            for fb in range(C.lwins // FW):
                src2 = _make_ap(red_d[:], fb * FW * 128 * 68,
                                [[FW * 68, 128], [1, FW * 68]])
                rt = fp.tile([128, FW, 68], F32, tag="rt")
                nc.sync.dma_start(
                    out=rt[:].rearrange("p w e -> p (w e)"), in_=src2)
                rec = fp.tile([128, FW, 4], F32, tag="rec")
                nc.vector.tensor_scalar_add(out=rec[:], in0=rt[:, :, 64:68],
                                            scalar1=EPS)
                nc.vector.reciprocal(out=rec[:], in_=rec[:])
                ot = fp.tile([128, FW, 64], F32, tag="ot")
                nc.vector.tensor_mul(
                    out=ot[:].rearrange("p k (h d) -> p k h d", d=16),
                    in0=rt[:, :, 0:64].rearrange("p k (h d) -> p k h d",
                                                 d=16),
                    in1=rec[:].to_broadcast([128, FW, 4, 16]))
                dst2 = _make_ap(out_d[:], fb * FW * 128 * 64,
                                [[FW * 64, 128], [1, FW * 64]])
                nc.sync.dma_start(
                    out=dst2, in_=ot[:].rearrange("p w e -> p (w e)"))

    nc.compile()
    return nc


def _build_base(C, B, num_devices=None):
    """I/O-identical near-empty program for dispatch-overhead calibration."""
    ND = num_devices or C.cores
    tcols = C.wins * B
    nc = bacc.Bacc("TRN2", target_bir_lowering=False, debug=False,
                   enable_asserts=False, num_devices=ND)
    nc.dram_tensor("xT", [IN_CH, C.nloc], F32, kind="ExternalInput")
    nc.dram_tensor("rhs_ext", [IN_CH, 72], F32, kind="ExternalInput")
    io_d = nc.dram_tensor("iota", [128, 128], F32, kind="ExternalInput")
    nc.dram_tensor("ident", [128, 128], F16, kind="ExternalInput")
    nc.dram_tensor("idxp", [128, tcols * 8], I16, kind="ExternalInput")
    nc.dram_tensor("dstc", [128, tcols], F16, kind="ExternalInput")
    nc.dram_tensor("wc", [128, tcols], F16, kind="ExternalInput")
    out_d = nc.dram_tensor("out", [C.nloc, 64], F32, kind="ExternalOutput")
    with tile.TileContext(nc) as tc, ExitStack() as ctx:
        sb = ctx.enter_context(tc.tile_pool(name="sb", bufs=1))
        f = sb.tile([128, 64], F32)
        nc.sync.dma_start(out=f[:], in_=io_d[:, 0:64])
        dst = _make_ap(out_d[:], 0, [[64, 128], [0, C.lwins], [1, 64]])
        nc.sync.dma_start(out=dst, in_=_bcast_dim(f[:], 1, C.lwins))
    nc.compile()
    return nc


def _unpermute_out(C, arr):
    """[12544, 64] block-permuted -> node order within the core's rows."""
    nb = C.lwins // C.chw
    return arr.reshape(nb, 128, C.chw, 64).transpose(
        0, 2, 1, 3).reshape(C.nloc, 64)


def _assemble(C, parts):
    """Per-core [12544, 64] outputs -> full [100352, 64] node order."""
    nb = C.lwins // C.chw               # blocks per core (14)
    bs = C.chw * 128                    # block rows (896)
    full = np.empty((C.npad, 64), dtype=parts[0].dtype)
    segrs = os.environ.get("K2_SEGRS", "1") == "1"
    for k in range(C.cores):
        blk = parts[k].reshape(nb, 128, C.chw, 64).transpose(0, 2, 1, 3)
        for s in range(nb):
            g = 8 * s + k if segrs else k * nb + s
            full[g * bs:(g + 1) * bs] = blk[s].reshape(bs, 64)
    return full


def kernel(x, edge_index, edge_weight, W, a):
    global LAST_EXEC_NS, LAST_NC, LAST_IN_MAPS
    C = Cfg()
    t0 = time.time()
    in_maps, B = _host_prep(C, x, edge_index, edge_weight, W, a)
    t1 = time.time()
    nc = _build_program(C, B)
    LAST_NC = nc
    LAST_IN_MAPS = in_maps
    t2 = time.time()
    res = bass_utils.run_bass_kernel_spmd(
        nc, in_maps, core_ids=list(range(C.cores)))
    t3 = time.time()
    print(f"[kernel2] host_prep {t1-t0:.1f}s  build+compile {t2-t1:.1f}s  "
          f"exec(all-in) {t3-t2:.1f}s  B={B}")
    LAST_EXEC_NS = res.exec_time_ns
    parts = [res.results[c]["out"] for c in range(C.cores)]
    full = _assemble(C, parts)[:N_NODES]
    return np.ascontiguousarray(full)
